# revision 11
# baseline (speedup 1.0000x reference)
"""RealFormer-style MultiHeadAttention on 8 Trainium2 NeuronCores.

Reference computation (B=8, S=1024, D=1024, H=16, HD=64):
    q = split_heads(hidden @ Wq + bq); k = ...; v = ...
    scores = (q @ k^T) * HD**-0.5 + attn_mask + prev_attn_weights
    out    = merge_heads(softmax(scores) @ v)

Sharding: pure data-parallel over batch - one batch element per core,
no collectives.

Per-core kernel design (matmul operands fp16, accumulation fp32):
  * Host folds SCALE into Wq, attn_mask into prev, pre-transposes hidden
    and prev ([h,k,q]), pre-interleaves Wq/Wk into per-head-pair slices,
    and casts everything streamed to fp16.
  * Projections are software-pipelined with head processing: q/k columns
    for head pair t are projected right before that pair's scores, so
    ScalarE is never idle behind a monolithic projection phase.  v is
    projected in two 8-head column chunks.
  * Per head pair (2t, 2t+1): PE copies prev^T k-tiles into PSUM via an
    identity matmul (start=True), then scoresT[k,q] = kT^T @ qT
    accumulates on top.  The two heads' K=64 score matmuls are issued
    back-to-back so they run concurrently in disjoint row-halves of the
    128x128 PE array.
  * Pool engine drains scoresT PSUM tiles to a per-head fp16 SBUF stage;
    ScalarE then runs ONE exp() over the whole head (N=8192, in place),
    amortizing per-instruction overhead ~8x vs per-tile exp.
    exp(s - 10) keeps fp16 range; the shift cancels in normalization.
  * vx[S, H*65] has a ones column per head so PV also produces softmax
    row-sums.  ctxT[65, q] stays transposed: the kernel ships raw
    ctxT+sums per head (fp16) and the HOST does the divide + head-merge
    transpose - no PE transposes, no on-chip normalization.
"""

import sys

if "/opt/trn_rl_repo" not in sys.path:
    sys.path.insert(0, "/opt/trn_rl_repo")

import numpy as np

B, S, D, H = 8, 1024, 1024, 16
HD = D // H
SCALE = HD**-0.5
P = 128
N_CORES = 8
EXP_SHIFT = 10.0

_compiled = {}


def _build(use_bias: bool, reps: int = 1):
    import concourse.bacc as bacc
    import concourse.mybir as mybir
    import concourse.tile as tile
    from concourse.masks import make_identity

    f16 = mybir.dt.float16
    f32 = mybir.dt.float32
    f8 = mybir.dt.float8e4
    Exp = mybir.ActivationFunctionType.Exp
    DoubleRow = mybir.MatmulPerfMode.DoubleRow

    nc = bacc.Bacc("TRN2", target_bir_lowering=False, debug=False)

    hT_d = nc.dram_tensor("hiddenT", (D, S), f16, kind="ExternalInput").ap()
    # wqk[t, ki, ko, j, col]: head-pair t's Wq (j=0) / Wk (j=1) columns,
    # k-tile-major - contiguous 4KB per partition line per pair.
    wqk_d = nc.dram_tensor("wqk", (8, P, 8, 2, P), f16, kind="ExternalInput").ap()
    # wv[ki, ko, d]: k-tile-major Wv
    wv_d = nc.dram_tensor("wv", (P, 8, D), f16, kind="ExternalInput").ap()
    # prevm8[h, ki, ko, j, q]: prev^T split into fp8e4m3 hi (j=0) + residual
    # lo (j=1), k-tile-major - consumed by a DoubleRow identity matmul that
    # computes hi+lo at 2 rows/cycle.  Same bytes as fp16, ~2x inject rate.
    prev_d = nc.dram_tensor("prevm8", (H, P, 8, 2, S), f8, kind="ExternalInput").ap()
    b_d = {}
    if use_bias:
        b_d = {
            name: nc.dram_tensor(name, (1, D), f16, kind="ExternalInput").ap()
            for name in ("bq", "bk", "bv")
        }
    # Unnormalized ctxT + sums per head: outc[h, e, q]; e==64 is the
    # softmax denominator row.  Host divides + merges heads.
    outc_d = nc.dram_tensor("outc", (H, 65, S), f16, kind="ExternalOutput").ap()

    with tile.TileContext(nc) as tc:
        with (
            tc.tile_pool(name="big", bufs=1) as big,
            tc.tile_pool(name="wqk", bufs=3) as wqk_pool,
            tc.tile_pool(name="qkt", bufs=3) as qkt_pool,
            tc.tile_pool(name="ppool", bufs=3) as ppool,
            tc.tile_pool(name="probs", bufs=4) as probs_pool,
            tc.tile_pool(name="ctxsb", bufs=3) as ctx_pool,
            tc.tile_pool(name="const", bufs=1) as const_pool,
            tc.tile_pool(name="ps_main", bufs=3, space="PSUM") as ps_main,
            tc.tile_pool(name="ps_ctx", bufs=2, space="PSUM") as ps_ctx,
        ):
            for _rep in range(reps):
                ident8 = const_pool.tile([P, 2, P], f8)
                nc.gpsimd.memset(ident8, 0.0)
                for j in range(2):
                    make_identity(nc, ident8[:, j, :], nomemset=True)
                neg_shift = const_pool.tile([P, 1], f32)
                nc.any.memset(neg_shift, -EXP_SHIFT)
                if use_bias:
                    ones_row = const_pool.tile([1, 512], f16)
                    nc.any.memset(ones_row, 1.0)
                    b_sb = {}
                    for name in ("bq", "bk", "bv"):
                        bt = const_pool.tile([1, D], f16, name=f"bsb_{name}")
                        nc.sync.dma_start(bt, b_d[name])
                        b_sb[name] = bt

                hidT = big.tile([P, 8, S], f16, tag="hidT")
                nc.sync.dma_start(hidT, hT_d.rearrange("(do di) s -> di do s", di=P))

                vx = big.tile([P, 8, H * 65], f16, tag="vx")
                vx_view = vx.rearrange("p t (h c) -> p t h c", c=65)
                nc.any.memset(vx_view[:, :, :, 64], 1.0)

                wqk_live = {}

                def emit_wqk_dma(t):
                    wt = wqk_pool.tile([P, 8, 2, P], f16, tag="wqk", name=f"wqk_{t}")
                    nc.sync.dma_start(wt, wqk_d[t])
                    wqk_live[t] = wt

                prev_live = {}

                def emit_prev_dma(h):
                    pj = ppool.tile([P, 8, 2, S], f8, tag="prev", name=f"prev_{h}")
                    nc.sync.dma_start(pj, prev_d[h])
                    prev_live[h] = pj

                qkT_live = {}

                def emit_qk_proj(t):
                    # project q/k output dims [128t .. 128t+127] -> qkT[:, j, :]
                    wt = wqk_live.pop(t)
                    dest = qkt_pool.tile([P, 2, S], f16, tag="qkT", name=f"qkT_{t}")
                    qkT_live[t] = dest
                    for j, pname in ((0, "q"), (1, "k")):
                        pt = ps_main.tile([P, S], f32, tag="ps", name=f"ps_{pname}{t}")
                        for half in range(2):
                            hs = slice(half * 512, half * 512 + 512)
                            for kt in range(8):
                                nc.tensor.matmul(
                                    pt[:, hs],
                                    lhsT=wt[:, kt, j, :],
                                    rhs=hidT[:, kt, hs],
                                    start=(kt == 0),
                                    stop=(kt == 7 and not use_bias),
                                )
                            if use_bias:
                                nc.tensor.matmul(
                                    pt[:, hs],
                                    lhsT=b_sb["b" + pname][:, t * P : (t + 1) * P],
                                    rhs=ones_row,
                                    start=False,
                                    stop=True,
                                )
                        nc.vector.tensor_copy(dest[:, j, :], pt[:])

                def emit_v_proj(chunk):
                    # v columns [512*chunk .. 512*chunk+511] (heads 8c..8c+7)
                    hs = slice(chunk * 512, chunk * 512 + 512)
                    for pt_i in range(8):
                        pv = ps_main.tile([P, S], f32, tag="ps", name=f"ps_v{chunk}{pt_i}")
                        for dt in range(8):
                            nc.tensor.matmul(
                                pv[:, 0:512],
                                lhsT=hidT[:, dt, pt_i * P : (pt_i + 1) * P],
                                rhs=wv_sb[:, dt, hs],
                                start=(dt == 0),
                                stop=(dt == 7 and not use_bias),
                            )
                        if use_bias:
                            nc.tensor.matmul(
                                pv[:, 0:512],
                                lhsT=ones_row[:, :P],
                                rhs=b_sb["bv"][:, hs],
                                start=False,
                                stop=True,
                            )
                        nc.vector.tensor_copy(
                            vx_view[:, pt_i, 8 * chunk : 8 * chunk + 8, 0:64],
                            pv[:, 0:512].rearrange("p (h e) -> p h e", e=64),
                        )

                probsT_live = {}

                def emit_scores(t):
                    hA, hB = 2 * t, 2 * t + 1
                    prevA, prevB = prev_live[hA], prev_live[hB]
                    qk = qkT_live.pop(t)
                    stA = probs_pool.tile([P, 8, S], f16, tag="probsT", name=f"pr_{hA}")
                    stB = probs_pool.tile([P, 8, S], f16, tag="probsT", name=f"pr_{hB}")
                    probsT_live[hA], probsT_live[hB] = stA, stB
                    for kt in range(8):
                        ks = slice(kt * P, (kt + 1) * P)
                        psA = ps_main.tile([P, S], f32, tag="ps", name=f"ps_s{hA}_{kt}")
                        psB = ps_main.tile([P, S], f32, tag="ps", name=f"ps_s{hB}_{kt}")
                        for ps, pj in ((psA, prevA), (psB, prevB)):
                            for half in range(2):
                                hs = slice(half * 512, half * 512 + 512)
                                nc.tensor.matmul(
                                    ps[:, hs],
                                    lhsT=ident8[:, 0:2, :],
                                    rhs=pj[:, kt, 0:2, hs],
                                    start=True,
                                    stop=False,
                                    perf_mode=DoubleRow,
                                    skip_group_check=True,
                                )
                        # paired K=64 score matmuls: back-to-back per half so
                        # they run concurrently in disjoint PE row-halves
                        for half in range(2):
                            hs = slice(half * 512, half * 512 + 512)
                            nc.tensor.matmul(
                                psA[:, hs],
                                lhsT=qk[0:64, 1, ks],
                                rhs=qk[0:64, 0, hs],
                                start=False,
                                stop=True,
                                skip_group_check=True,
                            )
                            nc.tensor.matmul(
                                psB[:, hs],
                                lhsT=qk[64:128, 1, ks],
                                rhs=qk[64:128, 0, hs],
                                start=False,
                                stop=True,
                                skip_group_check=True,
                            )
                        # exp straight out of PSUM into the fp16 probsT tiles
                        nc.scalar.activation(stA[:, kt, :], psA[:], Exp, bias=neg_shift)
                        nc.scalar.activation(stB[:, kt, :], psB[:], Exp, bias=neg_shift)

                def emit_ctx(t):
                    for h in (2 * t, 2 * t + 1):
                        probsT = probsT_live.pop(h)
                        prev_live.pop(h, None)
                        outc_sb = ctx_pool.tile([65, S], f16, tag="ctxT", name=f"ct_{h}")
                        for half in range(2):
                            hs = slice(half * 512, half * 512 + 512)
                            pc = ps_ctx.tile(
                                [65, 512], f32, tag="psc", name=f"ps_c{h}{half}"
                            )
                            for kt in range(8):
                                nc.tensor.matmul(
                                    pc,
                                    lhsT=vx[:, kt, h * 65 : (h + 1) * 65],
                                    rhs=probsT[:, kt, hs],
                                    start=(kt == 0),
                                    stop=(kt == 7),
                                )
                            nc.vector.tensor_copy(outc_sb[:, hs], pc)
                        nc.sync.dma_start(outc_d[h], outc_sb)

                # ---- schedule (DMA emission order = SP-queue priority) ----
                emit_wqk_dma(0)
                emit_prev_dma(0)
                emit_prev_dma(1)
                emit_wqk_dma(1)
                wv_sb = big.tile([P, 8, D], f16, tag="wv")
                nc.sync.dma_start(wv_sb, wv_d)
                emit_prev_dma(2)

                emit_qk_proj(0)
                emit_scores(0)
                emit_v_proj(0)
                for t in range(1, 8):
                    emit_qk_proj(t)
                    if t < 7:
                        emit_wqk_dma(t + 1)
                    for h in (2 * t + 1, 2 * t + 2):
                        if 2 < h < 16:
                            emit_prev_dma(h)
                    emit_scores(t)
                    emit_ctx(t - 1)
                    if t == 4:
                        emit_v_proj(1)
                emit_ctx(7)

    nc.compile()
    return nc


def _get_compiled(use_bias: bool, reps: int = 1):
    key = (use_bias, reps)
    if key not in _compiled:
        _compiled[key] = _build(use_bias, reps)
    return _compiled[key]


def _prepare_in_maps(
    hidden_states, attn_mask, prev_attn_weights, Wq, bq, Wk, bk, Wv, bv, use_bias
):
    hs = np.asarray(hidden_states, np.float32)
    mask = np.asarray(attn_mask, np.float32)
    prev = np.asarray(prev_attn_weights, np.float32)

    wq16 = (np.asarray(Wq, np.float32) * SCALE).astype(np.float16)
    wk16 = np.asarray(Wk, np.float32).astype(np.float16)
    wv16 = np.asarray(Wv, np.float32).astype(np.float16)

    # wqk[t, ki, ko, j, col]: pair t's Wq/Wk columns, k-tile-major
    wqk = np.empty((8, P, 8, 2, P), np.float16)
    for t in range(8):
        cs = slice(t * P, (t + 1) * P)
        wqk[t, :, :, 0, :] = wq16[:, cs].reshape(8, P, P).transpose(1, 0, 2)
        wqk[t, :, :, 1, :] = wk16[:, cs].reshape(8, P, P).transpose(1, 0, 2)
    # wv[ki, ko, d]
    wvr = np.ascontiguousarray(wv16.reshape(8, P, D).transpose(1, 0, 2))

    # fold mask in, pre-transpose to [b, h, k, q], split fp8 hi + residual lo,
    # lay out k-tile-major interleaved: [b, h, ki, ko, j, q]
    import ml_dtypes

    f8 = ml_dtypes.float8_e4m3
    if np.any(mask):
        prevT = (prev + mask).transpose(0, 1, 3, 2)
    else:
        prevT = prev.transpose(0, 1, 3, 2)
    prevT = prevT.reshape(B, H, 8, P, S).transpose(0, 1, 3, 2, 4)  # [b,h,ki,ko,q]
    hi = prevT.astype(f8)
    lo = (prevT - hi.astype(np.float32)).astype(f8)
    prevm8 = np.stack([hi, lo], axis=4)  # [b, h, ki, ko, 2, q]
    hT = np.ascontiguousarray(hs.transpose(0, 2, 1)).astype(np.float16)

    in_maps = []
    for b in range(N_CORES):
        m = {
            "hiddenT": np.ascontiguousarray(hT[b]),
            "wqk": wqk,
            "wv": wvr,
            "prevm8": np.ascontiguousarray(prevm8[b]),
        }
        if use_bias:
            m["bq"] = (np.asarray(bq, np.float32) * SCALE).astype(np.float16)[None, :]
            m["bk"] = np.asarray(bk, np.float32).astype(np.float16)[None, :]
            m["bv"] = np.asarray(bv, np.float32).astype(np.float16)[None, :]
        in_maps.append(m)
    return in_maps


def _finish_host(outc):
    # outc: [B, H, 65, S] fp16 -> out [B, S, D] fp32
    outc = outc.astype(np.float32)
    ctx = outc[:, :, 0:64, :]  # [B, H, 64, S]
    denom = outc[:, :, 64:65, :]  # [B, H, 1, S]
    ctx = ctx / denom
    # [B, H, 64, S] -> [B, S, H*64]
    return np.ascontiguousarray(ctx.transpose(0, 3, 1, 2).reshape(B, S, D))


def kernel(hidden_states, attn_mask, prev_attn_weights, Wq, bq, Wk, bk, Wv, bv):
    from concourse.bass_utils import run_bass_kernel_spmd

    use_bias = bool(np.any(bq) or np.any(bk) or np.any(bv))
    nc = _get_compiled(use_bias)
    in_maps = _prepare_in_maps(
        hidden_states, attn_mask, prev_attn_weights, Wq, bq, Wk, bk, Wv, bv, use_bias
    )
    res = run_bass_kernel_spmd(nc, in_maps, core_ids=list(range(N_CORES)))
    outc = np.stack([res.results[b]["outc"] for b in range(N_CORES)])
    return _finish_host(outc)


# revision 20
# speedup vs baseline: 1.0424x; 1.0424x over previous
"""RealFormer-style MultiHeadAttention on 8 Trainium2 NeuronCores.

Reference computation (B=8, S=1024, D=1024, H=16, HD=64):
    q = split_heads(hidden @ Wq + bq); k = ...; v = ...
    scores = (q @ k^T) * HD**-0.5 + attn_mask + prev_attn_weights
    out    = merge_heads(softmax(scores) @ v)

Sharding: pure data-parallel over batch - one batch element per core,
no collectives.

Per-core kernel design (matmul operands fp16, accumulation fp32):
  * Host folds SCALE into Wq, attn_mask into prev, pre-transposes hidden
    and prev ([h,k,q]), pre-interleaves Wq/Wk into per-head-pair slices,
    and casts everything streamed to fp16.
  * Projections are software-pipelined with head processing: q/k columns
    for head pair t are projected right before that pair's scores, so
    ScalarE is never idle behind a monolithic projection phase.  v is
    projected in two 8-head column chunks.
  * Per head pair (2t, 2t+1): PE copies prev^T k-tiles into PSUM via an
    identity matmul (start=True), then scoresT[k,q] = kT^T @ qT
    accumulates on top.  The two heads' K=64 score matmuls are issued
    back-to-back so they run concurrently in disjoint row-halves of the
    128x128 PE array.
  * Pool engine drains scoresT PSUM tiles to a per-head fp16 SBUF stage;
    ScalarE then runs ONE exp() over the whole head (N=8192, in place),
    amortizing per-instruction overhead ~8x vs per-tile exp.
    exp(s - 10) keeps fp16 range; the shift cancels in normalization.
  * vx[S, H*65] has a ones column per head so PV also produces softmax
    row-sums.  ctxT[65, q] stays transposed: the kernel ships raw
    ctxT+sums per head (fp16) and the HOST does the divide + head-merge
    transpose - no PE transposes, no on-chip normalization.
"""

import os
import sys

if "/opt/trn_rl_repo" not in sys.path:
    sys.path.insert(0, "/opt/trn_rl_repo")

import numpy as np

# timing-only probes (produce WRONG results; never set when grading):
#   noexp2  - emit exp for even k-tiles only (halves ScalarE work)
#   noinject - skip the prev DoubleRow injection matmuls (cuts PE work)
#   nopv2   - PV accumulates only even k-tiles (cuts PE work)
_PROBE = os.environ.get("KERNEL_PROBE", "")

B, S, D, H = 8, 1024, 1024, 16
HD = D // H
SCALE = HD**-0.5
P = 128
N_CORES = 8
EXP_SHIFT = 10.0

_compiled = {}


def _build(use_bias: bool, reps: int = 1):
    import concourse.bacc as bacc
    import concourse.mybir as mybir
    import concourse.tile as tile
    from concourse.masks import make_identity

    f16 = mybir.dt.float16
    f32 = mybir.dt.float32
    f8 = mybir.dt.float8e4
    Exp = mybir.ActivationFunctionType.Exp
    DoubleRow = mybir.MatmulPerfMode.DoubleRow

    nc = bacc.Bacc("TRN2", target_bir_lowering=False, debug=False)

    hT_d = nc.dram_tensor("hiddenT", (D, S), f16, kind="ExternalInput").ap()
    # wqk[t, ki, ko, j, col]: head-pair t's Wq (j=0) / Wk (j=1) columns,
    # k-tile-major - contiguous 4KB per partition line per pair.
    wqk_d = nc.dram_tensor("wqk", (8, P, 8, 2, P), f16, kind="ExternalInput").ap()
    # wv[ki, ko, d]: k-tile-major Wv
    wv_d = nc.dram_tensor("wv", (P, 8, D), f16, kind="ExternalInput").ap()
    # prevp8[t, ki, ko, h2, j, q]: prev^T for head pair t, split into fp8e4m3
    # hi (j=0) + residual lo (j=1), k-tile-major - consumed by a DoubleRow
    # identity matmul that computes hi+lo at 2 rows/cycle.  Same bytes as
    # fp16, ~2x inject rate; one 4MB DMA per pair (dma_start issue overhead
    # is ~2-3us each, so fewer+bigger transfers win).
    prev_d = nc.dram_tensor(
        "prevp8", (8, P, 8, 2, 2, S), f8, kind="ExternalInput"
    ).ap()
    b_d = {}
    if use_bias:
        b_d = {
            name: nc.dram_tensor(name, (1, D), f16, kind="ExternalInput").ap()
            for name in ("bq", "bk", "bv")
        }
    # Unnormalized ctxT + sums per head: outc[h, e, q]; e==64 is the
    # softmax denominator row.  Host divides + merges heads.
    outc_d = nc.dram_tensor("outc", (H, 65, S), f16, kind="ExternalOutput").ap()

    with tile.TileContext(nc) as tc:
        with (
            tc.tile_pool(name="big", bufs=1) as big,
            tc.tile_pool(name="wqk", bufs=3) as wqk_pool,
            tc.tile_pool(name="qkt", bufs=3) as qkt_pool,
            tc.tile_pool(name="ppool", bufs=3) as ppool,
            tc.tile_pool(name="probs", bufs=4) as probs_pool,
            tc.tile_pool(name="ctxsb", bufs=3) as ctx_pool,
            tc.tile_pool(name="const", bufs=1) as const_pool,
            tc.tile_pool(name="ps_main", bufs=3, space="PSUM") as ps_main,
            tc.tile_pool(name="ps_ctx", bufs=2, space="PSUM") as ps_ctx,
        ):
            for _rep in range(reps):
                ident8 = const_pool.tile([P, 2, P], f8)
                nc.gpsimd.memset(ident8, 0.0)
                for j in range(2):
                    make_identity(nc, ident8[:, j, :], nomemset=True)
                neg_shift = const_pool.tile([P, 1], f32)
                nc.any.memset(neg_shift, -EXP_SHIFT)
                if use_bias:
                    ones_row = const_pool.tile([1, 512], f16)
                    nc.any.memset(ones_row, 1.0)
                    b_sb = {}
                    for name in ("bq", "bk", "bv"):
                        bt = const_pool.tile([1, D], f16, name=f"bsb_{name}")
                        nc.sync.dma_start(bt, b_d[name])
                        b_sb[name] = bt

                hidT = big.tile([P, 8, S], f16, tag="hidT")
                nc.sync.dma_start(hidT, hT_d.rearrange("(do di) s -> di do s", di=P))

                vx = big.tile([P, 8, H * 65], f16, tag="vx")
                vx_view = vx.rearrange("p t (h c) -> p t h c", c=65)
                nc.any.memset(vx_view[:, :, :, 64], 1.0)

                wqk_live = {}

                def emit_wqk_dma(t):
                    wt = wqk_pool.tile([P, 8, 2, P], f16, tag="wqk", name=f"wqk_{t}")
                    nc.sync.dma_start(wt, wqk_d[t])
                    wqk_live[t] = wt

                prev_live = {}

                def emit_prev_dma(t):
                    pj = ppool.tile([P, 8, 2, 2, S], f8, tag="prev", name=f"prev_{t}")
                    nc.sync.dma_start(pj, prev_d[t])
                    prev_live[t] = pj

                qkT_live = {}

                def emit_qk_proj(t):
                    # project q/k output dims [128t .. 128t+127] -> qkT[:, j, :]
                    wt = wqk_live.pop(t)
                    dest = qkt_pool.tile([P, 2, S], f16, tag="qkT", name=f"qkT_{t}")
                    qkT_live[t] = dest
                    for j, pname in ((0, "q"), (1, "k")):
                        pt = ps_main.tile([P, S], f32, tag="ps", name=f"ps_{pname}{t}")
                        for half in range(2):
                            hs = slice(half * 512, half * 512 + 512)
                            for kt in range(8):
                                nc.tensor.matmul(
                                    pt[:, hs],
                                    lhsT=wt[:, kt, j, :],
                                    rhs=hidT[:, kt, hs],
                                    start=(kt == 0),
                                    stop=(kt == 7 and not use_bias),
                                )
                            if use_bias:
                                nc.tensor.matmul(
                                    pt[:, hs],
                                    lhsT=b_sb["b" + pname][:, t * P : (t + 1) * P],
                                    rhs=ones_row,
                                    start=False,
                                    stop=True,
                                )
                        nc.vector.tensor_copy(dest[:, j, :], pt[:])

                def emit_v_proj(chunk):
                    # v columns [512*chunk .. 512*chunk+511] (heads 8c..8c+7)
                    hs = slice(chunk * 512, chunk * 512 + 512)
                    for pt_i in range(8):
                        pv = ps_main.tile([P, S], f32, tag="ps", name=f"ps_v{chunk}{pt_i}")
                        for dt in range(8):
                            nc.tensor.matmul(
                                pv[:, 0:512],
                                lhsT=hidT[:, dt, pt_i * P : (pt_i + 1) * P],
                                rhs=wv_sb[:, dt, hs],
                                start=(dt == 0),
                                stop=(dt == 7 and not use_bias),
                            )
                        if use_bias:
                            nc.tensor.matmul(
                                pv[:, 0:512],
                                lhsT=ones_row[:, :P],
                                rhs=b_sb["bv"][:, hs],
                                start=False,
                                stop=True,
                            )
                        nc.vector.tensor_copy(
                            vx_view[:, pt_i, 8 * chunk : 8 * chunk + 8, 0:64],
                            pv[:, 0:512].rearrange("p (h e) -> p h e", e=64),
                        )

                probsT_live = {}

                def emit_scores(t):
                    hA, hB = 2 * t, 2 * t + 1
                    pj = prev_live[t]
                    qk = qkT_live.pop(t)
                    stA = probs_pool.tile([P, 8, S], f16, tag="probsT", name=f"pr_{hA}")
                    stB = probs_pool.tile([P, 8, S], f16, tag="probsT", name=f"pr_{hB}")
                    probsT_live[hA], probsT_live[hB] = stA, stB
                    for kt in range(8):
                        ks = slice(kt * P, (kt + 1) * P)
                        psA = ps_main.tile([P, S], f32, tag="ps", name=f"ps_s{hA}_{kt}")
                        psB = ps_main.tile([P, S], f32, tag="ps", name=f"ps_s{hB}_{kt}")
                        if _PROBE != "noinject":
                            for ps, pj in ((psA, prevA), (psB, prevB)):
                                for half in range(2):
                                    hs = slice(half * 512, half * 512 + 512)
                                    nc.tensor.matmul(
                                        ps[:, hs],
                                        lhsT=ident8[:, 0:2, :],
                                        rhs=pj[:, kt, 0:2, hs],
                                        start=True,
                                        stop=False,
                                        perf_mode=DoubleRow,
                                        skip_group_check=True,
                                    )
                        # paired K=64 score matmuls: back-to-back per half so
                        # they run concurrently in disjoint PE row-halves
                        for half in range(2):
                            hs = slice(half * 512, half * 512 + 512)
                            nc.tensor.matmul(
                                psA[:, hs],
                                lhsT=qk[0:64, 1, ks],
                                rhs=qk[0:64, 0, hs],
                                start=False,
                                stop=True,
                                skip_group_check=True,
                            )
                            nc.tensor.matmul(
                                psB[:, hs],
                                lhsT=qk[64:128, 1, ks],
                                rhs=qk[64:128, 0, hs],
                                start=False,
                                stop=True,
                                skip_group_check=True,
                            )
                        # exp straight out of PSUM into the fp16 probsT tiles
                        if _PROBE != "noexp2" or kt % 2 == 0:
                            nc.scalar.activation(
                                stA[:, kt, :], psA[:], Exp, bias=neg_shift
                            )
                            nc.scalar.activation(
                                stB[:, kt, :], psB[:], Exp, bias=neg_shift
                            )

                def emit_ctx(t):
                    for h in (2 * t, 2 * t + 1):
                        probsT = probsT_live.pop(h)
                        prev_live.pop(h, None)
                        outc_sb = ctx_pool.tile([65, S], f16, tag="ctxT", name=f"ct_{h}")
                        for half in range(2):
                            hs = slice(half * 512, half * 512 + 512)
                            pc = ps_ctx.tile(
                                [65, 512], f32, tag="psc", name=f"ps_c{h}{half}"
                            )
                            kts = range(0, 8, 2) if _PROBE == "nopv2" else range(8)
                            last = list(kts)[-1]
                            for kt in kts:
                                nc.tensor.matmul(
                                    pc,
                                    lhsT=vx[:, kt, h * 65 : (h + 1) * 65],
                                    rhs=probsT[:, kt, hs],
                                    start=(kt == 0),
                                    stop=(kt == last),
                                )
                            nc.vector.tensor_copy(outc_sb[:, hs], pc)
                        nc.sync.dma_start(outc_d[h], outc_sb)

                # ---- schedule (DMA emission order = SP-queue priority) ----
                emit_wqk_dma(0)
                emit_prev_dma(0)
                emit_prev_dma(1)
                emit_wqk_dma(1)
                wv_sb = big.tile([P, 8, D], f16, tag="wv")
                nc.sync.dma_start(wv_sb, wv_d)
                emit_prev_dma(2)

                emit_qk_proj(0)
                emit_scores(0)
                emit_v_proj(0)
                for t in range(1, 8):
                    emit_qk_proj(t)
                    if t < 7:
                        emit_wqk_dma(t + 1)
                    for h in (2 * t + 1, 2 * t + 2):
                        if 2 < h < 16:
                            emit_prev_dma(h)
                    emit_scores(t)
                    emit_ctx(t - 1)
                    if t == 4:
                        emit_v_proj(1)
                emit_ctx(7)

    nc.compile()
    return nc


def _get_compiled(use_bias: bool, reps: int = 1):
    key = (use_bias, reps)
    if key not in _compiled:
        _compiled[key] = _build(use_bias, reps)
    return _compiled[key]


def _prepare_in_maps(
    hidden_states, attn_mask, prev_attn_weights, Wq, bq, Wk, bk, Wv, bv, use_bias
):
    hs = np.asarray(hidden_states, np.float32)
    mask = np.asarray(attn_mask, np.float32)
    prev = np.asarray(prev_attn_weights, np.float32)

    wq16 = (np.asarray(Wq, np.float32) * SCALE).astype(np.float16)
    wk16 = np.asarray(Wk, np.float32).astype(np.float16)
    wv16 = np.asarray(Wv, np.float32).astype(np.float16)

    # wqk[t, ki, ko, j, col]: pair t's Wq/Wk columns, k-tile-major
    wqk = np.empty((8, P, 8, 2, P), np.float16)
    for t in range(8):
        cs = slice(t * P, (t + 1) * P)
        wqk[t, :, :, 0, :] = wq16[:, cs].reshape(8, P, P).transpose(1, 0, 2)
        wqk[t, :, :, 1, :] = wk16[:, cs].reshape(8, P, P).transpose(1, 0, 2)
    # wv[ki, ko, d]
    wvr = np.ascontiguousarray(wv16.reshape(8, P, D).transpose(1, 0, 2))

    # fold mask in, pre-transpose to [b, h, k, q], split fp8 hi + residual lo,
    # lay out k-tile-major interleaved: [b, h, ki, ko, j, q]
    import ml_dtypes

    f8 = ml_dtypes.float8_e4m3
    if np.any(mask):
        prevT = (prev + mask).transpose(0, 1, 3, 2)
    else:
        prevT = prev.transpose(0, 1, 3, 2)
    prevT = prevT.reshape(B, H, 8, P, S).transpose(0, 1, 3, 2, 4)  # [b,h,ki,ko,q]
    hi = prevT.astype(f8)
    lo = (prevT - hi.astype(np.float32)).astype(f8)
    prevm8 = np.stack([hi, lo], axis=4)  # [b, h, ki, ko, 2, q]
    hT = np.ascontiguousarray(hs.transpose(0, 2, 1)).astype(np.float16)

    in_maps = []
    for b in range(N_CORES):
        m = {
            "hiddenT": np.ascontiguousarray(hT[b]),
            "wqk": wqk,
            "wv": wvr,
            "prevm8": np.ascontiguousarray(prevm8[b]),
        }
        if use_bias:
            m["bq"] = (np.asarray(bq, np.float32) * SCALE).astype(np.float16)[None, :]
            m["bk"] = np.asarray(bk, np.float32).astype(np.float16)[None, :]
            m["bv"] = np.asarray(bv, np.float32).astype(np.float16)[None, :]
        in_maps.append(m)
    return in_maps


def _finish_host(outc):
    # outc: [B, H, 65, S] fp16 -> out [B, S, D] fp32
    outc = outc.astype(np.float32)
    ctx = outc[:, :, 0:64, :]  # [B, H, 64, S]
    denom = outc[:, :, 64:65, :]  # [B, H, 1, S]
    ctx = ctx / denom
    # [B, H, 64, S] -> [B, S, H*64]
    return np.ascontiguousarray(ctx.transpose(0, 3, 1, 2).reshape(B, S, D))


def kernel(hidden_states, attn_mask, prev_attn_weights, Wq, bq, Wk, bk, Wv, bv):
    from concourse.bass_utils import run_bass_kernel_spmd

    use_bias = bool(np.any(bq) or np.any(bk) or np.any(bv))
    nc = _get_compiled(use_bias)
    in_maps = _prepare_in_maps(
        hidden_states, attn_mask, prev_attn_weights, Wq, bq, Wk, bk, Wv, bv, use_bias
    )
    res = run_bass_kernel_spmd(nc, in_maps, core_ids=list(range(N_CORES)))
    outc = np.stack([res.results[b]["outc"] for b in range(N_CORES)])
    return _finish_host(outc)


# revision 27
# speedup vs baseline: 1.0898x; 1.0455x over previous
"""RealFormer-style MultiHeadAttention on 8 Trainium2 NeuronCores.

Reference computation (B=8, S=1024, D=1024, H=16, HD=64):
    q = split_heads(hidden @ Wq + bq); k = ...; v = ...
    scores = (q @ k^T) * HD**-0.5 + attn_mask + prev_attn_weights
    out    = merge_heads(softmax(scores) @ v)

Sharding: pure data-parallel over batch - one batch element per core,
no collectives.

Per-core kernel design (matmul operands fp16, accumulation fp32):
  * Host folds SCALE into Wq, attn_mask into prev, pre-transposes hidden
    and prev ([h,k,q]), pre-interleaves Wq/Wk into per-head-pair slices,
    and casts everything streamed to fp16.
  * Projections are software-pipelined with head processing: q/k columns
    for head pair t are projected right before that pair's scores, so
    ScalarE is never idle behind a monolithic projection phase.  v is
    projected in two 8-head column chunks.
  * Per head pair (2t, 2t+1): PE copies prev^T k-tiles into PSUM via an
    identity matmul (start=True), then scoresT[k,q] = kT^T @ qT
    accumulates on top.  The two heads' K=64 score matmuls are issued
    back-to-back so they run concurrently in disjoint row-halves of the
    128x128 PE array.
  * Pool engine drains scoresT PSUM tiles to a per-head fp16 SBUF stage;
    ScalarE then runs ONE exp() over the whole head (N=8192, in place),
    amortizing per-instruction overhead ~8x vs per-tile exp.
    exp(s - 10) keeps fp16 range; the shift cancels in normalization.
  * vx[S, H*65] has a ones column per head so PV also produces softmax
    row-sums.  ctxT[65, q] stays transposed: the kernel ships raw
    ctxT+sums per head (fp16) and the HOST does the divide + head-merge
    transpose - no PE transposes, no on-chip normalization.
"""

import sys

if "/opt/trn_rl_repo" not in sys.path:
    sys.path.insert(0, "/opt/trn_rl_repo")

import numpy as np

B, S, D, H = 8, 1024, 1024, 16
HD = D // H
SCALE = HD**-0.5
P = 128
N_CORES = 8
EXP_SHIFT = 10.0

_compiled = {}


def _build(use_bias: bool, reps: int = 1):
    import concourse.bacc as bacc
    import concourse.mybir as mybir
    import concourse.tile as tile
    from concourse.masks import make_identity

    f16 = mybir.dt.float16
    f32 = mybir.dt.float32
    f8 = mybir.dt.float8e4
    Exp = mybir.ActivationFunctionType.Exp
    DoubleRow = mybir.MatmulPerfMode.DoubleRow

    nc = bacc.Bacc("TRN2", target_bir_lowering=False, debug=False)

    hT_d = nc.dram_tensor("hiddenT", (D, S), f16, kind="ExternalInput").ap()
    # wqk[t, ki, ko, j, col]: head-pair t's Wq (j=0) / Wk (j=1) columns,
    # k-tile-major - contiguous 4KB per partition line per pair.
    wqk_d = nc.dram_tensor("wqk", (8, P, 8, 2, P), f16, kind="ExternalInput").ap()
    # wv[ki, ko, d]: k-tile-major Wv
    wv_d = nc.dram_tensor("wv", (P, 8, D), f16, kind="ExternalInput").ap()
    # prevp8[t, ki, ko, h2, j, q]: prev^T for head pair t, split into fp8e4m3
    # hi (j=0) + residual lo (j=1), k-tile-major - consumed by a DoubleRow
    # identity matmul that computes hi+lo at 2 rows/cycle.  Same bytes as
    # fp16, ~2x inject rate; one 4MB DMA per pair (dma_start issue overhead
    # is ~2-3us each, so fewer+bigger transfers win).
    prev_d = nc.dram_tensor(
        "prevp8", (8, P, 8, 2, 2, S), f8, kind="ExternalInput"
    ).ap()
    b_d = {}
    if use_bias:
        b_d = {
            name: nc.dram_tensor(name, (1, D), f16, kind="ExternalInput").ap()
            for name in ("bq", "bk", "bv")
        }
    # Unnormalized ctxT + sums per head: outc[h, e, q]; e==64 is the
    # softmax denominator row.  Host divides + merges heads.
    outc_d = nc.dram_tensor("outc", (H, 65, S), f16, kind="ExternalOutput").ap()

    with tile.TileContext(nc) as tc:
        with (
            tc.tile_pool(name="big", bufs=1) as big,
            tc.tile_pool(name="wqk", bufs=3) as wqk_pool,
            tc.tile_pool(name="qkt", bufs=3) as qkt_pool,
            tc.tile_pool(name="ppool", bufs=2) as ppool,
            tc.tile_pool(name="probs", bufs=3) as probs_pool,
            tc.tile_pool(name="ctxsb", bufs=2) as ctx_pool,
            tc.tile_pool(name="const", bufs=1) as const_pool,
            tc.tile_pool(name="ps_main", bufs=3, space="PSUM") as ps_main,
            tc.tile_pool(name="ps_ctx", bufs=2, space="PSUM") as ps_ctx,
        ):
            for _rep in range(reps):
                ident8 = const_pool.tile([P, 2, P], f8)
                nc.gpsimd.memset(ident8, 0.0)
                for j in range(2):
                    make_identity(nc, ident8[:, j, :], nomemset=True)
                neg_shift = const_pool.tile([P, 1], f32)
                nc.any.memset(neg_shift, -EXP_SHIFT)
                if use_bias:
                    ones_row = const_pool.tile([1, 512], f16)
                    nc.any.memset(ones_row, 1.0)
                    b_sb = {}
                    for name in ("bq", "bk", "bv"):
                        bt = const_pool.tile([1, D], f16, name=f"bsb_{name}")
                        nc.sync.dma_start(bt, b_d[name])
                        b_sb[name] = bt

                hidT = big.tile([P, 8, S], f16, tag="hidT")
                nc.sync.dma_start(hidT, hT_d.rearrange("(do di) s -> di do s", di=P))

                vx = big.tile([P, 8, H * 65], f16, tag="vx")
                vx_view = vx.rearrange("p t (h c) -> p t h c", c=65)
                nc.any.memset(vx_view[:, :, :, 64], 1.0)

                wqk_live = {}

                def emit_wqk_dma(t):
                    wt = wqk_pool.tile([P, 8, 2, P], f16, tag="wqk", name=f"wqk_{t}")
                    nc.sync.dma_start(wt, wqk_d[t])
                    wqk_live[t] = wt

                prev_live = {}

                def emit_prev_dma(t):
                    pj = ppool.tile([P, 8, 2, 2, S], f8, tag="prev", name=f"prev_{t}")
                    nc.sync.dma_start(pj, prev_d[t])
                    prev_live[t] = pj

                qkT_live = {}

                def emit_qk_proj(t):
                    # project q/k output dims [128t .. 128t+127] -> qkT[:, j, :]
                    wt = wqk_live.pop(t)
                    dest = qkt_pool.tile([P, 2, S], f16, tag="qkT", name=f"qkT_{t}")
                    qkT_live[t] = dest
                    for j, pname in ((0, "q"), (1, "k")):
                        pt = ps_main.tile([P, S], f32, tag="ps", name=f"ps_{pname}{t}")
                        for half in range(2):
                            hs = slice(half * 512, half * 512 + 512)
                            for kt in range(8):
                                nc.tensor.matmul(
                                    pt[:, hs],
                                    lhsT=wt[:, kt, j, :],
                                    rhs=hidT[:, kt, hs],
                                    start=(kt == 0),
                                    stop=(kt == 7 and not use_bias),
                                )
                            if use_bias:
                                nc.tensor.matmul(
                                    pt[:, hs],
                                    lhsT=b_sb["b" + pname][:, t * P : (t + 1) * P],
                                    rhs=ones_row,
                                    start=False,
                                    stop=True,
                                )
                        nc.vector.tensor_copy(dest[:, j, :], pt[:])

                def emit_v_proj(chunk):
                    # v columns [512*chunk .. 512*chunk+511] (heads 8c..8c+7)
                    hs = slice(chunk * 512, chunk * 512 + 512)
                    for pt_i in range(8):
                        pv = ps_main.tile([P, S], f32, tag="ps", name=f"ps_v{chunk}{pt_i}")
                        for dt in range(8):
                            nc.tensor.matmul(
                                pv[:, 0:512],
                                lhsT=hidT[:, dt, pt_i * P : (pt_i + 1) * P],
                                rhs=wv_sb[:, dt, hs],
                                start=(dt == 0),
                                stop=(dt == 7 and not use_bias),
                            )
                        if use_bias:
                            nc.tensor.matmul(
                                pv[:, 0:512],
                                lhsT=ones_row[:, :P],
                                rhs=b_sb["bv"][:, hs],
                                start=False,
                                stop=True,
                            )
                        nc.vector.tensor_copy(
                            vx_view[:, pt_i, 8 * chunk : 8 * chunk + 8, 0:64],
                            pv[:, 0:512].rearrange("p (h e) -> p h e", e=64),
                        )

                probsT_live = {}

                def emit_scores(t):
                    hA, hB = 2 * t, 2 * t + 1
                    pj = prev_live[t]
                    qk = qkT_live.pop(t)
                    stA = probs_pool.tile([P, 8, S], f16, tag="probsT", name=f"pr_{hA}")
                    stB = probs_pool.tile([P, 8, S], f16, tag="probsT", name=f"pr_{hB}")
                    probsT_live[hA], probsT_live[hB] = stA, stB
                    for kt in range(8):
                        ks = slice(kt * P, (kt + 1) * P)
                        psA = ps_main.tile([P, S], f32, tag="ps", name=f"ps_s{hA}_{kt}")
                        psB = ps_main.tile([P, S], f32, tag="ps", name=f"ps_s{hB}_{kt}")
                        for ps, h2 in ((psA, 0), (psB, 1)):
                            for half in range(2):
                                hs = slice(half * 512, half * 512 + 512)
                                nc.tensor.matmul(
                                    ps[:, hs],
                                    lhsT=ident8[:, 0:2, :],
                                    rhs=pj[:, kt, h2, 0:2, hs],
                                    start=True,
                                    stop=False,
                                    perf_mode=DoubleRow,
                                    skip_group_check=True,
                                )
                        # paired K=64 score matmuls: back-to-back per half so
                        # they run concurrently in disjoint PE row-halves
                        for half in range(2):
                            hs = slice(half * 512, half * 512 + 512)
                            nc.tensor.matmul(
                                psA[:, hs],
                                lhsT=qk[0:64, 1, ks],
                                rhs=qk[0:64, 0, hs],
                                start=False,
                                stop=True,
                                skip_group_check=True,
                            )
                            nc.tensor.matmul(
                                psB[:, hs],
                                lhsT=qk[64:128, 1, ks],
                                rhs=qk[64:128, 0, hs],
                                start=False,
                                stop=True,
                                skip_group_check=True,
                            )
                        # exp straight out of PSUM into the fp16 probsT tiles
                        nc.scalar.activation(stA[:, kt, :], psA[:], Exp, bias=neg_shift)
                        nc.scalar.activation(stB[:, kt, :], psB[:], Exp, bias=neg_shift)

                outc_group = [None]

                def emit_ctx(t):
                    for h in (2 * t, 2 * t + 1):
                        probsT = probsT_live.pop(h)
                        if h % 4 == 0:
                            outc_group[0] = ctx_pool.tile(
                                [65, 4, S], f16, tag="ctxT", name=f"ct_{h // 4}"
                            )
                        outc_sb = outc_group[0]
                        for half in range(2):
                            hs = slice(half * 512, half * 512 + 512)
                            pc = ps_ctx.tile(
                                [65, 512], f32, tag="psc", name=f"ps_c{h}{half}"
                            )
                            for kt in range(8):
                                nc.tensor.matmul(
                                    pc,
                                    lhsT=vx[:, kt, h * 65 : (h + 1) * 65],
                                    rhs=probsT[:, kt, hs],
                                    start=(kt == 0),
                                    stop=(kt == 7),
                                )
                            nc.vector.tensor_copy(outc_sb[:, h % 4, hs], pc)
                        if h % 4 == 3:
                            g = h // 4
                            nc.sync.dma_start(
                                outc_d[4 * g : 4 * g + 4].rearrange("h e q -> e h q"),
                                outc_sb,
                            )
                    prev_live.pop(t, None)

                # ---- schedule (DMA emission order = SP-queue priority) ----
                emit_wqk_dma(0)
                emit_prev_dma(0)
                emit_wqk_dma(1)
                wv_sb = big.tile([P, 8, D], f16, tag="wv")
                nc.sync.dma_start(wv_sb, wv_d)
                emit_prev_dma(1)

                emit_qk_proj(0)
                emit_scores(0)
                emit_v_proj(0)
                for t in range(1, 8):
                    emit_qk_proj(t)
                    if t < 7:
                        emit_wqk_dma(t + 1)
                        emit_prev_dma(t + 1)
                    emit_scores(t)
                    emit_ctx(t - 1)
                    if t == 4:
                        emit_v_proj(1)
                emit_ctx(7)

    nc.compile()
    return nc


def _get_compiled(use_bias: bool, reps: int = 1):
    key = (use_bias, reps)
    if key not in _compiled:
        _compiled[key] = _build(use_bias, reps)
    return _compiled[key]


def _prepare_in_maps(
    hidden_states, attn_mask, prev_attn_weights, Wq, bq, Wk, bk, Wv, bv, use_bias
):
    hs = np.asarray(hidden_states, np.float32)
    mask = np.asarray(attn_mask, np.float32)
    prev = np.asarray(prev_attn_weights, np.float32)

    wq16 = (np.asarray(Wq, np.float32) * SCALE).astype(np.float16)
    wk16 = np.asarray(Wk, np.float32).astype(np.float16)
    wv16 = np.asarray(Wv, np.float32).astype(np.float16)

    # wqk[t, ki, ko, j, col]: pair t's Wq/Wk columns, k-tile-major
    wqk = np.empty((8, P, 8, 2, P), np.float16)
    for t in range(8):
        cs = slice(t * P, (t + 1) * P)
        wqk[t, :, :, 0, :] = wq16[:, cs].reshape(8, P, P).transpose(1, 0, 2)
        wqk[t, :, :, 1, :] = wk16[:, cs].reshape(8, P, P).transpose(1, 0, 2)
    # wv[ki, ko, d]
    wvr = np.ascontiguousarray(wv16.reshape(8, P, D).transpose(1, 0, 2))

    # fold mask in, pre-transpose to [b, h, k, q], split fp8 hi + residual lo,
    # lay out k-tile-major interleaved: [b, h, ki, ko, j, q]
    import ml_dtypes

    f8 = ml_dtypes.float8_e4m3
    if np.any(mask):
        prevT = (prev + mask).transpose(0, 1, 3, 2)
    else:
        prevT = prev.transpose(0, 1, 3, 2)
    prevT = prevT.reshape(B, H, 8, P, S).transpose(0, 1, 3, 2, 4)  # [b,h,ki,ko,q]
    hi = prevT.astype(f8)
    lo = (prevT - hi.astype(np.float32)).astype(f8)
    prevm8 = np.stack([hi, lo], axis=4)  # [b, h, ki, ko, j, q]
    # pair-major: [b, t, ki, ko, h2, j, q]
    prevp8 = prevm8.reshape(B, 8, 2, P, 8, 2, S).transpose(0, 1, 3, 4, 2, 5, 6)
    hT = np.ascontiguousarray(hs.transpose(0, 2, 1)).astype(np.float16)

    in_maps = []
    for b in range(N_CORES):
        m = {
            "hiddenT": np.ascontiguousarray(hT[b]),
            "wqk": wqk,
            "wv": wvr,
            "prevp8": np.ascontiguousarray(prevp8[b]),
        }
        if use_bias:
            m["bq"] = (np.asarray(bq, np.float32) * SCALE).astype(np.float16)[None, :]
            m["bk"] = np.asarray(bk, np.float32).astype(np.float16)[None, :]
            m["bv"] = np.asarray(bv, np.float32).astype(np.float16)[None, :]
        in_maps.append(m)
    return in_maps


def _finish_host(outc):
    # outc: [B, H, 65, S] fp16 -> out [B, S, D] fp32
    outc = outc.astype(np.float32)
    ctx = outc[:, :, 0:64, :]  # [B, H, 64, S]
    denom = outc[:, :, 64:65, :]  # [B, H, 1, S]
    ctx = ctx / denom
    # [B, H, 64, S] -> [B, S, H*64]
    return np.ascontiguousarray(ctx.transpose(0, 3, 1, 2).reshape(B, S, D))


def kernel(hidden_states, attn_mask, prev_attn_weights, Wq, bq, Wk, bk, Wv, bv):
    from concourse.bass_utils import run_bass_kernel_spmd

    use_bias = bool(np.any(bq) or np.any(bk) or np.any(bv))
    nc = _get_compiled(use_bias)
    in_maps = _prepare_in_maps(
        hidden_states, attn_mask, prev_attn_weights, Wq, bq, Wk, bk, Wv, bv, use_bias
    )
    res = run_bass_kernel_spmd(nc, in_maps, core_ids=list(range(N_CORES)))
    outc = np.stack([res.results[b]["outc"] for b in range(N_CORES)])
    return _finish_host(outc)


# revision 33
# speedup vs baseline: 1.1276x; 1.0346x over previous
"""RealFormer-style MultiHeadAttention on 8 Trainium2 NeuronCores.

Reference computation (B=8, S=1024, D=1024, H=16, HD=64):
    q = split_heads(hidden @ Wq + bq); k = ...; v = ...
    scores = (q @ k^T) * HD**-0.5 + attn_mask + prev_attn_weights
    out    = merge_heads(softmax(scores) @ v)

Sharding: pure data-parallel over batch - one batch element per core,
no collectives.

Per-core kernel design (matmul operands fp16/fp8, accumulation fp32):
  * Host folds SCALE into Wq, attn_mask into prev, pre-transposes hidden
    and prev ([h,k,q]), pre-interleaves Wq/Wk into per-head-pair slices,
    and casts the streamed operands to fp16 (fp8 hi+lo for prev).
  * Projections are software-pipelined with head processing: q/k columns
    for head pair t are projected right before that pair's scores, so
    ScalarE is never idle behind a monolithic projection phase.  v is
    projected in two 8-head column chunks.
  * Per head pair (2t, 2t+1): prev^T is shipped as fp8e4m3 hi + residual
    lo pairs and injected into PSUM by ONE DoubleRow identity matmul
    (computes hi+lo at 2 rows/cycle - same bytes as fp16, ~2x the inject
    rate, ~1e-3 total error).  scoresT[k,q] = kT^T @ qT accumulates on
    top.  The two heads' K=64 score matmuls are issued back-to-back so
    they run concurrently in disjoint row-halves of the 128x128 PE array.
  * ScalarE exps each scores tile straight out of PSUM into fp16 probsT;
    exp(s - 10) keeps fp16 range; the shift cancels in normalization.
  * DMAs are few and large (one 4MB transfer per head pair's prev, output
    in 4-head groups) - each dma_start carries ~2-3us issue overhead.
  * vx[S, H*65] has a ones column per head so PV also produces softmax
    row-sums.  ctxT[65, q] stays transposed: the kernel ships raw
    ctxT+sums per head (fp16) and the HOST does the divide + head-merge
    transpose - no PE transposes, no on-chip normalization.
"""

import sys

if "/opt/trn_rl_repo" not in sys.path:
    sys.path.insert(0, "/opt/trn_rl_repo")

import numpy as np

B, S, D, H = 8, 1024, 1024, 16
HD = D // H
SCALE = HD**-0.5
P = 128
N_CORES = 8
EXP_SHIFT = 10.0

_compiled = {}


def _build(use_bias: bool, reps: int = 1):
    import concourse.bacc as bacc
    import concourse.mybir as mybir
    import concourse.tile as tile
    from concourse.masks import make_identity

    f16 = mybir.dt.float16
    f32 = mybir.dt.float32
    f8 = mybir.dt.float8e4
    Exp = mybir.ActivationFunctionType.Exp
    DoubleRow = mybir.MatmulPerfMode.DoubleRow

    nc = bacc.Bacc("TRN2", target_bir_lowering=False, debug=False)

    hT_d = nc.dram_tensor("hiddenT", (D, S), f16, kind="ExternalInput").ap()
    # wqk[t, ki, ko, j, col]: head-pair t's Wq (j=0) / Wk (j=1) columns,
    # k-tile-major - contiguous 4KB per partition line per pair.
    wqk_d = nc.dram_tensor("wqk", (8, P, 8, 2, P), f16, kind="ExternalInput").ap()
    # wv[ki, ko, d]: k-tile-major Wv
    wv_d = nc.dram_tensor("wv", (P, 8, D), f16, kind="ExternalInput").ap()
    # prevp8[t, ki, ko, h2, j, q]: prev^T for head pair t, split into fp8e4m3
    # hi (j=0) + residual lo (j=1), k-tile-major - consumed by a DoubleRow
    # identity matmul that computes hi+lo at 2 rows/cycle.  Same bytes as
    # fp16, ~2x inject rate; one 4MB DMA per pair (dma_start issue overhead
    # is ~2-3us each, so fewer+bigger transfers win).
    prev_d = nc.dram_tensor(
        "prevp8", (8, P, 8, 2, 2, S), f8, kind="ExternalInput"
    ).ap()
    b_d = {}
    if use_bias:
        b_d = {
            name: nc.dram_tensor(name, (1, D), f16, kind="ExternalInput").ap()
            for name in ("bq", "bk", "bv")
        }
    # Unnormalized ctxT + sums per head: outc[h, e, q]; e==64 is the
    # softmax denominator row.  Host divides + merges heads.
    outc_d = nc.dram_tensor("outc", (H, 65, S), f16, kind="ExternalOutput").ap()

    with tile.TileContext(nc) as tc:
        with (
            tc.tile_pool(name="big", bufs=1) as big,
            tc.tile_pool(name="wqk", bufs=3) as wqk_pool,
            tc.tile_pool(name="qkt", bufs=3) as qkt_pool,
            tc.tile_pool(name="ppool", bufs=2) as ppool,
            tc.tile_pool(name="probs", bufs=3) as probs_pool,
            tc.tile_pool(name="ctxsb", bufs=2) as ctx_pool,
            tc.tile_pool(name="const", bufs=1) as const_pool,
            tc.tile_pool(name="ps_main", bufs=3, space="PSUM") as ps_main,
            tc.tile_pool(name="ps_ctx", bufs=2, space="PSUM") as ps_ctx,
        ):
            for _rep in range(reps):
                ident8 = const_pool.tile([P, 2, P], f8)
                nc.gpsimd.memset(ident8, 0.0)
                for j in range(2):
                    make_identity(nc, ident8[:, j, :], nomemset=True)
                neg_shift = const_pool.tile([P, 1], f32)
                nc.any.memset(neg_shift, -EXP_SHIFT)
                if use_bias:
                    ones_row = const_pool.tile([1, 512], f16)
                    nc.any.memset(ones_row, 1.0)
                    b_sb = {}
                    for name in ("bq", "bk", "bv"):
                        bt = const_pool.tile([1, D], f16, name=f"bsb_{name}")
                        nc.sync.dma_start(bt, b_d[name])
                        b_sb[name] = bt

                hidT = big.tile([P, 8, S], f16, tag="hidT")
                nc.sync.dma_start(hidT, hT_d.rearrange("(do di) s -> di do s", di=P))

                vx = big.tile([P, 8, H * 65], f16, tag="vx")
                vx_view = vx.rearrange("p t (h c) -> p t h c", c=65)
                nc.any.memset(vx_view[:, :, :, 64], 1.0)

                wqk_live = {}

                def emit_wqk_dma(t):
                    wt = wqk_pool.tile([P, 8, 2, P], f16, tag="wqk", name=f"wqk_{t}")
                    nc.sync.dma_start(wt, wqk_d[t])
                    wqk_live[t] = wt

                prev_live = {}

                def emit_prev_dma(t, split=False):
                    pj = ppool.tile([P, 8, 2, 2, S], f8, tag="prev", name=f"prev_{t}")
                    if split:
                        # per-head chunks so pair 0's first scores can start
                        # ~5us earlier during the ramp
                        nc.sync.dma_start(pj[:, :, 0], prev_d[t][:, :, 0])
                        nc.sync.dma_start(pj[:, :, 1], prev_d[t][:, :, 1])
                    else:
                        nc.sync.dma_start(pj, prev_d[t])
                    prev_live[t] = pj

                qkT_live = {}

                def emit_qk_proj(t):
                    # project q/k output dims [128t .. 128t+127] -> qkT[:, j, :]
                    wt = wqk_live.pop(t)
                    dest = qkt_pool.tile([P, 2, S], f16, tag="qkT", name=f"qkT_{t}")
                    qkT_live[t] = dest
                    for j, pname in ((0, "q"), (1, "k")):
                        pt = ps_main.tile([P, S], f32, tag="ps", name=f"ps_{pname}{t}")
                        # kt-outer so each stationary W chunk is loaded once
                        # and serves both moving halves
                        for kt in range(8):
                            for half in range(2):
                                hs = slice(half * 512, half * 512 + 512)
                                nc.tensor.matmul(
                                    pt[:, hs],
                                    lhsT=wt[:, kt, j, :],
                                    rhs=hidT[:, kt, hs],
                                    start=(kt == 0),
                                    stop=(kt == 7 and not use_bias),
                                    skip_group_check=True,
                                )
                        if use_bias:
                            for half in range(2):
                                hs = slice(half * 512, half * 512 + 512)
                                nc.tensor.matmul(
                                    pt[:, hs],
                                    lhsT=b_sb["b" + pname][:, t * P : (t + 1) * P],
                                    rhs=ones_row,
                                    start=False,
                                    stop=True,
                                    skip_group_check=True,
                                )
                        nc.vector.tensor_copy(dest[:, j, :], pt[:])

                def emit_v_proj(chunk):
                    # v columns [512*chunk .. 512*chunk+511] (heads 8c..8c+7)
                    hs = slice(chunk * 512, chunk * 512 + 512)
                    for pt_i in range(8):
                        pv = ps_main.tile([P, S], f32, tag="ps", name=f"ps_v{chunk}{pt_i}")
                        for dt in range(8):
                            nc.tensor.matmul(
                                pv[:, 0:512],
                                lhsT=hidT[:, dt, pt_i * P : (pt_i + 1) * P],
                                rhs=wv_sb[:, dt, hs],
                                start=(dt == 0),
                                stop=(dt == 7 and not use_bias),
                            )
                        if use_bias:
                            nc.tensor.matmul(
                                pv[:, 0:512],
                                lhsT=ones_row[:, :P],
                                rhs=b_sb["bv"][:, hs],
                                start=False,
                                stop=True,
                            )
                        nc.vector.tensor_copy(
                            vx_view[:, pt_i, 8 * chunk : 8 * chunk + 8, 0:64],
                            pv[:, 0:512].rearrange("p (h e) -> p h e", e=64),
                        )

                probsT_live = {}

                def emit_scores(t):
                    hA, hB = 2 * t, 2 * t + 1
                    pj = prev_live[t]
                    qk = qkT_live.pop(t)
                    stA = probs_pool.tile([P, 8, S], f16, tag="probsT", name=f"pr_{hA}")
                    stB = probs_pool.tile([P, 8, S], f16, tag="probsT", name=f"pr_{hB}")
                    probsT_live[hA], probsT_live[hB] = stA, stB
                    for kt in range(8):
                        ks = slice(kt * P, (kt + 1) * P)
                        psA = ps_main.tile([P, S], f32, tag="ps", name=f"ps_s{hA}_{kt}")
                        psB = ps_main.tile([P, S], f32, tag="ps", name=f"ps_s{hB}_{kt}")
                        for ps, h2 in ((psA, 0), (psB, 1)):
                            for half in range(2):
                                hs = slice(half * 512, half * 512 + 512)
                                nc.tensor.matmul(
                                    ps[:, hs],
                                    lhsT=ident8[:, 0:2, :],
                                    rhs=pj[:, kt, h2, 0:2, hs],
                                    start=True,
                                    stop=False,
                                    perf_mode=DoubleRow,
                                    skip_group_check=True,
                                )
                        # paired K=64 score matmuls: each head's stationary
                        # kT serves both halves back-to-back (one weight
                        # load), and the B-strip matmuls overlap the A-strip
                        # ones in the disjoint row-half of the PE array
                        for ps, rr in ((psA, slice(0, 64)), (psB, slice(64, 128))):
                            for half in range(2):
                                hs = slice(half * 512, half * 512 + 512)
                                nc.tensor.matmul(
                                    ps[:, hs],
                                    lhsT=qk[rr, 1, ks],
                                    rhs=qk[rr, 0, hs],
                                    start=False,
                                    stop=True,
                                    skip_group_check=True,
                                )
                        # exp straight out of PSUM into the fp16 probsT tiles
                        nc.scalar.activation(stA[:, kt, :], psA[:], Exp, bias=neg_shift)
                        nc.scalar.activation(stB[:, kt, :], psB[:], Exp, bias=neg_shift)

                outc_group = [None]

                def emit_ctx(t):
                    for h in (2 * t, 2 * t + 1):
                        probsT = probsT_live.pop(h)
                        if h % 4 == 0:
                            outc_group[0] = ctx_pool.tile(
                                [65, 4, S], f16, tag="ctxT", name=f"ct_{h // 4}"
                            )
                        outc_sb = outc_group[0]
                        for half in range(2):
                            hs = slice(half * 512, half * 512 + 512)
                            pc = ps_ctx.tile(
                                [65, 512], f32, tag="psc", name=f"ps_c{h}{half}"
                            )
                            for kt in range(8):
                                nc.tensor.matmul(
                                    pc,
                                    lhsT=vx[:, kt, h * 65 : (h + 1) * 65],
                                    rhs=probsT[:, kt, hs],
                                    start=(kt == 0),
                                    stop=(kt == 7),
                                )
                            nc.vector.tensor_copy(outc_sb[:, h % 4, hs], pc)
                        if h % 4 == 3:
                            g = h // 4
                            nc.sync.dma_start(
                                outc_d[4 * g : 4 * g + 4].rearrange("h e q -> e h q"),
                                outc_sb,
                            )
                    prev_live.pop(t, None)

                # ---- schedule (DMA emission order = SP-queue priority) ----
                emit_wqk_dma(0)
                emit_prev_dma(0, split=True)
                emit_wqk_dma(1)
                wv_sb = big.tile([P, 8, D], f16, tag="wv")
                nc.sync.dma_start(wv_sb, wv_d)
                emit_prev_dma(1)

                emit_qk_proj(0)
                emit_scores(0)
                emit_v_proj(0)
                for t in range(1, 8):
                    emit_qk_proj(t)
                    if t < 7:
                        emit_wqk_dma(t + 1)
                        emit_prev_dma(t + 1)
                    emit_scores(t)
                    emit_ctx(t - 1)
                    if t == 4:
                        emit_v_proj(1)
                emit_ctx(7)

    nc.compile()
    return nc


def _get_compiled(use_bias: bool, reps: int = 1):
    key = (use_bias, reps)
    if key not in _compiled:
        _compiled[key] = _build(use_bias, reps)
    return _compiled[key]


def _prepare_in_maps(
    hidden_states, attn_mask, prev_attn_weights, Wq, bq, Wk, bk, Wv, bv, use_bias
):
    hs = np.asarray(hidden_states, np.float32)
    mask = np.asarray(attn_mask, np.float32)
    prev = np.asarray(prev_attn_weights, np.float32)

    wq16 = (np.asarray(Wq, np.float32) * SCALE).astype(np.float16)
    wk16 = np.asarray(Wk, np.float32).astype(np.float16)
    wv16 = np.asarray(Wv, np.float32).astype(np.float16)

    # wqk[t, ki, ko, j, col]: pair t's Wq/Wk columns, k-tile-major
    wqk = np.empty((8, P, 8, 2, P), np.float16)
    for t in range(8):
        cs = slice(t * P, (t + 1) * P)
        wqk[t, :, :, 0, :] = wq16[:, cs].reshape(8, P, P).transpose(1, 0, 2)
        wqk[t, :, :, 1, :] = wk16[:, cs].reshape(8, P, P).transpose(1, 0, 2)
    # wv[ki, ko, d]
    wvr = np.ascontiguousarray(wv16.reshape(8, P, D).transpose(1, 0, 2))

    # fold mask in, pre-transpose to [b, h, k, q], split fp8 hi + residual lo,
    # lay out k-tile-major interleaved: [b, h, ki, ko, j, q]
    import ml_dtypes

    f8 = ml_dtypes.float8_e4m3
    if np.any(mask):
        prevT = (prev + mask).transpose(0, 1, 3, 2)
    else:
        prevT = prev.transpose(0, 1, 3, 2)
    prevT = prevT.reshape(B, H, 8, P, S).transpose(0, 1, 3, 2, 4)  # [b,h,ki,ko,q]
    hi = prevT.astype(f8)
    lo = (prevT - hi.astype(np.float32)).astype(f8)
    prevm8 = np.stack([hi, lo], axis=4)  # [b, h, ki, ko, j, q]
    # pair-major: [b, t, ki, ko, h2, j, q]
    prevp8 = prevm8.reshape(B, 8, 2, P, 8, 2, S).transpose(0, 1, 3, 4, 2, 5, 6)
    hT = np.ascontiguousarray(hs.transpose(0, 2, 1)).astype(np.float16)

    in_maps = []
    for b in range(N_CORES):
        m = {
            "hiddenT": np.ascontiguousarray(hT[b]),
            "wqk": wqk,
            "wv": wvr,
            "prevp8": np.ascontiguousarray(prevp8[b]),
        }
        if use_bias:
            m["bq"] = (np.asarray(bq, np.float32) * SCALE).astype(np.float16)[None, :]
            m["bk"] = np.asarray(bk, np.float32).astype(np.float16)[None, :]
            m["bv"] = np.asarray(bv, np.float32).astype(np.float16)[None, :]
        in_maps.append(m)
    return in_maps


def _finish_host(outc):
    # outc: [B, H, 65, S] fp16 -> out [B, S, D] fp32
    outc = outc.astype(np.float32)
    ctx = outc[:, :, 0:64, :]  # [B, H, 64, S]
    denom = outc[:, :, 64:65, :]  # [B, H, 1, S]
    ctx = ctx / denom
    # [B, H, 64, S] -> [B, S, H*64]
    return np.ascontiguousarray(ctx.transpose(0, 3, 1, 2).reshape(B, S, D))


def kernel(hidden_states, attn_mask, prev_attn_weights, Wq, bq, Wk, bk, Wv, bv):
    from concourse.bass_utils import run_bass_kernel_spmd

    use_bias = bool(np.any(bq) or np.any(bk) or np.any(bv))
    nc = _get_compiled(use_bias)
    in_maps = _prepare_in_maps(
        hidden_states, attn_mask, prev_attn_weights, Wq, bq, Wk, bk, Wv, bv, use_bias
    )
    res = run_bass_kernel_spmd(nc, in_maps, core_ids=list(range(N_CORES)))
    outc = np.stack([res.results[b]["outc"] for b in range(N_CORES)])
    return _finish_host(outc)
